# revision 4
# baseline (speedup 1.0000x reference)
"""Trainium2 Bass kernel for nn_EqStftSnsePBC (STFT -> per-tap nonlinear PBC -> ISTFT).

Strategy (8 NeuronCores, pure data parallelism over STFT frames):
  host:   frame the signal (hop 216, n_fft 256) into [stream, freq_in_frame, frame]
          fp32 arrays, shard 4632 (padded) frames as 579 per core; build DFT/IDFT
          matrices and the 256x256 wrap-around Toeplitz correlation matrix G
          (launch power P folded in).
  device: X = DFT(frames)        (fp32r matmuls, K=256 contracted in 2 chunks)
          I = sum_modes |X|^2    (DVE)
          phi' = G @ I           (fp32r matmuls; phi' = P * phi)
          Y = X * ((1 - phi'_i) + j phi'_r)   (GPSIMD elementwise)
          yf = IDFT(Y)           (fp32r matmuls, 1/256 folded into the matrix)
  host:   overlap-add yf frames, divide by coverage count, trim, restack.
"""

import os
import sys

for _p in ("/opt/trn_rl_repo",):
    if os.path.isdir(_p) and _p not in sys.path:
        sys.path.append(_p)

import numpy as np

# ---- problem geometry (hardcoded) ----
MTAPS = 41
PAD = MTAPS // 2  # 20
NFFT = 256
HOP = 216
B = 2
NM = 2
L = 999688
STEPS = 4628            # (L - NFFT) // HOP + 1
NCORES = 8
NH = 579                # frames per core
FTOT = NCORES * NH      # 4632 >= STEPS (4 trailing fake frames, ignored on host)
LOUT = L - 2 * PAD * 2  # L - overlaps = 999648
NBLOCKS = ((0, 290), (289, 290))   # (col offset, width); even widths >= 256 (fp32r ISA); 1-col overlap is benign
NSTREAMS = B * NM * 2   # (b, mode, re/im) -> 8

_PROG = None            # (nc, input_names) cache; compile once per process
LAST_EXEC_NS = None
LAST_RESULTS = None


def _build_const_matrices(h_real, h_imag, task_info):
    """DFT/IDFT lhsT matrices and per-batch P-scaled correlation matrices."""
    n = np.arange(NFFT)
    ang = 2.0 * np.pi * np.outer(n, n) / NFFT
    c, s = np.cos(ang), np.sin(ang)
    # forward lhsT[n, f] so that lhsT.T @ frames = W @ frames, W = exp(-2i pi f n / N)
    # stages: 0 fwd_r, 1 fwd_i, 2 fwd_minus_i, 3 inv_r, 4 inv_i, 5 inv_minus_i
    wmat = np.empty((12, 128, NFFT), np.float32)
    stages = [c, -s, s, c / NFFT, s / NFFT, -s / NFFT]
    for st, mat in enumerate(stages):
        wmat[st * 2 + 0] = mat[0:128, :].astype(np.float32)
        wmat[st * 2 + 1] = mat[128:256, :].astype(np.float32)

    # G[p', m] = sum of h[p-m] over p in [0,296) with (p-20) mod 256 == p'
    def toep(h):
        G = np.zeros((NFFT, NFFT), np.float64)
        for p in range(NFFT + 2 * PAD):
            pp = (p - PAD) % NFFT
            lo, hi = max(0, p - (MTAPS - 1)), min(NFFT - 1, p)
            if lo <= hi:
                ms = np.arange(lo, hi + 1)
                G[pp, ms] += h[p - ms]
        return G

    Gr, Gi = toep(np.asarray(h_real, np.float64)), toep(np.asarray(h_imag, np.float64))
    P = (10.0 ** (np.asarray(task_info, np.float64)[:, 0] / 10.0) / NM)
    gmat = np.empty((B, 4, 128, NFFT), np.float32)
    for b in range(B):
        for kc in range(2):
            gmat[b, 0 * 2 + kc] = (P[b] * Gr[kc * 128:(kc + 1) * 128, :]).astype(np.float32)
            gmat[b, 1 * 2 + kc] = (P[b] * Gi[kc * 128:(kc + 1) * 128, :]).astype(np.float32)
    return wmat, gmat


def _frame_inputs(x_real, x_imag):
    """-> xf [B, NM, 2, NFFT, FTOT] fp32: xf[b,m,ri,n,j] = x[b, HOP*j + n, m]."""
    need = HOP * (FTOT - 1) + NFFT
    xf = np.empty((B, NM, 2, NFFT, FTOT), np.float32)
    for ri, x in enumerate((x_real, x_imag)):
        xt = np.ascontiguousarray(np.asarray(x).transpose(0, 2, 1))  # [B, NM, L]
        xp = np.zeros((B, NM, need), np.float32)
        xp[:, :, :L] = xt
        sw = np.lib.stride_tricks.as_strided(
            xp,
            shape=(B, NM, FTOT, NFFT),
            strides=(xp.strides[0], xp.strides[1], HOP * 4, 4),
        )
        xf[:, :, ri] = sw.transpose(0, 1, 3, 2)
    return xf


def _overlap_add(yf):
    """yf [B, NM, 2, NFFT, FTOT] -> y [B, NM, 2, L] (OLA / coverage)."""
    y = np.zeros((B, NM, 2, STEPS, HOP), np.float32)
    body = yf[:, :, :, :HOP, :STEPS].transpose(0, 1, 2, 4, 3)  # [.., j, 216]
    y[:] = body
    tail = yf[:, :, :, HOP:, :STEPS - 1].transpose(0, 1, 2, 4, 3)  # [.., j, 40]
    y[:, :, :, 1:, :NFFT - HOP] += tail
    y = y.reshape(B, NM, 2, STEPS * HOP)
    yfull = np.empty((B, NM, 2, L), np.float32)
    yfull[:, :, :, :STEPS * HOP] = y
    yfull[:, :, :, STEPS * HOP:] = yf[:, :, :, HOP:, STEPS - 1]  # final tail, coverage 1
    # coverage: 2 on [216(j+1), 216(j+1)+40) for j=0..steps-2, else 1
    t = np.arange(L)
    wsum = np.ones(L, np.float32)
    wsum[(t >= HOP) & (t < STEPS * HOP) & (t % HOP < NFFT - HOP)] = 2.0
    yfull /= wsum
    return yfull


def _build_program(use_gpsimd=True, split_w=True):
    import concourse.bass as bass
    import concourse.tile as tile
    from concourse import bacc, mybir
    from contextlib import ExitStack

    f32 = mybir.dt.float32
    f32r = mybir.dt.float32r
    MULT = mybir.AluOpType.mult
    ADD = mybir.AluOpType.add
    SUB = mybir.AluOpType.subtract

    nc = bacc.Bacc(None, target_bir_lowering=False, debug=False)
    xf_d = nc.dram_tensor("xf", [NSTREAMS, NFFT, NH], f32, kind="ExternalInput").ap()
    wm_d = nc.dram_tensor("wmat", [12, 128, NFFT], f32, kind="ExternalInput").ap()
    gm_d = nc.dram_tensor("gmat", [B, 4, 128, NFFT], f32, kind="ExternalInput").ap()
    yf_d = nc.dram_tensor("yf", [NSTREAMS, NFFT, NH], f32, kind="ExternalOutput").ap()

    FWD_R, FWD_I, FWD_MI, INV_R, INV_I, INV_MI = range(6)

    with tile.TileContext(nc) as tc:
        with ExitStack() as ctx:
            consts = ctx.enter_context(tc.tile_pool(name="consts", bufs=1))
            xin = ctx.enter_context(tc.tile_pool(name="xin", bufs=2))
            xcp = ctx.enter_context(tc.tile_pool(name="xcp", bufs=2))
            work = ctx.enter_context(tc.tile_pool(name="work", bufs=2))
            ysb_p = ctx.enter_context(tc.tile_pool(name="ysb", bufs=2))
            osb_p = ctx.enter_context(tc.tile_pool(name="osb", bufs=2))
            ps_x = ctx.enter_context(tc.tile_pool(name="psx", bufs=4, space="PSUM"))
            ps_phi = ctx.enter_context(tc.tile_pool(name="psphi", bufs=2, space="PSUM"))
            ps_yf = ctx.enter_context(tc.tile_pool(name="psyf", bufs=2, space="PSUM"))

            def wslice(tile_or_pair, mh):
                if split_w:
                    return tile_or_pair[mh][:]
                return tile_or_pair[:, mh * 128:(mh + 1) * 128]

            wsb = []
            for t in range(12):
                if split_w:
                    pair = []
                    for mh in range(2):
                        w = consts.tile([128, 128], f32r, tag=f"w{t}_{mh}")
                        nc.sync.dma_start(w[:], wm_d[t, :, mh * 128:(mh + 1) * 128].bitcast(f32r))
                        pair.append(w)
                    wsb.append(pair)
                else:
                    w = consts.tile([128, NFFT], f32r, tag=f"w{t}")
                    nc.sync.dma_start(w[:], wm_d[t].bitcast(f32r))
                    wsb.append(w)
            gsb = [[None] * 4 for _ in range(B)]
            for b in range(B):
                for t in range(4):
                    if split_w:
                        pair = []
                        for mh in range(2):
                            g = consts.tile([128, 128], f32r, tag=f"g{b}_{t}_{mh}")
                            nc.sync.dma_start(g[:], gm_d[b, t, :, mh * 128:(mh + 1) * 128].bitcast(f32r))
                            pair.append(g)
                        gsb[b][t] = pair
                    else:
                        g = consts.tile([128, NFFT], f32r, tag=f"g{b}_{t}")
                        nc.sync.dma_start(g[:], gm_d[b, t].bitcast(f32r))
                        gsb[b][t] = g

            def sidx(b, m, ri):
                return b * 4 + m * 2 + ri

            for b in range(B):
                for (j0, NB) in NBLOCKS:
                    # ---- load frames ----
                    xsb = {}
                    for m in range(NM):
                        for ri in range(2):
                            for kc in range(2):
                                tl = xin.tile([128, NB], f32r, tag=f"x{m}{ri}{kc}")
                                nc.sync.dma_start(
                                    tl[:],
                                    xf_d[sidx(b, m, ri), kc * 128:(kc + 1) * 128,
                                         j0:j0 + NB].bitcast(f32r),
                                )
                                xsb[(m, ri, kc)] = tl

                    # ---- FFT + copy X to SBUF ----
                    FFT_TERMS = {0: ((FWD_R, 0), (FWD_MI, 1)),   # Xr = Wr xr - Wi xi
                                 1: ((FWD_R, 1), (FWD_I, 0))}    # Xi = Wr xi + Wi xr
                    Xsb = {}
                    for m in range(NM):
                        for ri_o in range(2):
                            for mh in range(2):
                                xp = ps_x.tile([128, NB], f32, tag="xps")
                                mms = [(st, src, kc)
                                       for (st, src) in FFT_TERMS[ri_o]
                                       for kc in range(2)]
                                for i, (st, src, kc) in enumerate(mms):
                                    nc.tensor.matmul(
                                        xp[:],
                                        wslice(wsb[st * 2 + kc], mh),
                                        xsb[(m, src, kc)][:],
                                        start=(i == 0), stop=(i == len(mms) - 1),
                                    )
                                xs = xcp.tile([128, NB], f32r, tag=f"X{m}{ri_o}{mh}")
                                nc.vector.tensor_copy(xs[:], xp[:])
                                Xsb[(m, ri_o, mh)] = xs

                    # ---- intensity I = sum_m (Xr^2 + Xi^2), per freq half ----
                    isb = {}
                    for mh in range(2):
                        it = work.tile([128, NB], f32r, tag=f"i{mh}")
                        t0 = work.tile([128, NB], f32r, tag=f"sqa{mh}")
                        t1 = work.tile([128, NB], f32r, tag=f"sqb{mh}")
                        nc.vector.tensor_tensor(t0[:], Xsb[(0, 0, mh)][:], Xsb[(0, 0, mh)][:], MULT)
                        nc.vector.tensor_tensor(t1[:], Xsb[(0, 1, mh)][:], Xsb[(0, 1, mh)][:], MULT)
                        nc.vector.tensor_tensor(it[:], t0[:], t1[:], ADD)
                        t2 = work.tile([128, NB], f32r, tag=f"sqc{mh}")
                        t3 = work.tile([128, NB], f32r, tag=f"sqd{mh}")
                        nc.vector.tensor_tensor(t2[:], Xsb[(1, 0, mh)][:], Xsb[(1, 0, mh)][:], MULT)
                        nc.vector.tensor_tensor(t3[:], Xsb[(1, 1, mh)][:], Xsb[(1, 1, mh)][:], MULT)
                        nc.vector.tensor_tensor(t2[:], t2[:], t3[:], ADD)
                        nc.vector.tensor_tensor(it[:], it[:], t2[:], ADD)
                        isb[mh] = it

                    # ---- phi' = P * corr(I, h); A = 1 - phi'_i, Bm = phi'_r ----
                    AB = {}
                    for ri in range(2):  # 0: phi'_r -> Bm ; 1: phi'_i -> A
                        for mh in range(2):
                            pp = ps_phi.tile([128, NB], f32, tag="phps")
                            for kc in range(2):
                                nc.tensor.matmul(
                                    pp[:],
                                    wslice(gsb[b][ri * 2 + kc], mh),
                                    isb[kc][:],
                                    start=(kc == 0), stop=(kc == 1),
                                )
                            ab = work.tile([128, NB], f32r, tag=f"ab{ri}{mh}")
                            if ri == 0:
                                nc.vector.tensor_copy(ab[:], pp[:])
                            else:
                                nc.vector.tensor_scalar(ab[:], pp[:], -1.0, 1.0, MULT, ADD)
                            AB[(ri, mh)] = ab

                    # ---- Y = X * (A + j Bm)  (gpsimd, SBUF only) ----
                    ysb = {}
                    for m in range(NM):
                        for mh in range(2):
                            A, Bm = AB[(1, mh)], AB[(0, mh)]
                            Xr, Xi = Xsb[(m, 0, mh)], Xsb[(m, 1, mh)]
                            t0 = work.tile([128, NB], f32r, tag=f"yt0{m}{mh}")
                            t1 = work.tile([128, NB], f32r, tag=f"yt1{m}{mh}")
                            yr = ysb_p.tile([128, NB], f32r, tag=f"yr{m}{mh}")
                            yi = ysb_p.tile([128, NB], f32r, tag=f"yi{m}{mh}")
                            eng = nc.gpsimd if use_gpsimd else nc.vector
                            eng.tensor_tensor(t0[:], Xr[:], A[:], MULT)
                            eng.tensor_tensor(t1[:], Xi[:], Bm[:], MULT)
                            eng.tensor_tensor(yr[:], t0[:], t1[:], SUB)
                            eng.tensor_tensor(t0[:], Xi[:], A[:], MULT)
                            eng.tensor_tensor(t1[:], Xr[:], Bm[:], MULT)
                            eng.tensor_tensor(yi[:], t0[:], t1[:], ADD)
                            ysb[(m, 0, mh)] = yr
                            ysb[(m, 1, mh)] = yi

                    # ---- IFFT + store ----
                    IFFT_TERMS = {0: ((INV_R, 0), (INV_MI, 1)),  # yfr = iWr Yr - iWi Yi
                                  1: ((INV_R, 1), (INV_I, 0))}   # yfi = iWr Yi + iWi Yr
                    for m in range(NM):
                        for ri_o in range(2):
                            for nh in range(2):
                                yp = ps_yf.tile([128, NB], f32, tag="yfps")
                                mms = [(st, src, kc)
                                       for (st, src) in IFFT_TERMS[ri_o]
                                       for kc in range(2)]
                                for i, (st, src, kc) in enumerate(mms):
                                    nc.tensor.matmul(
                                        yp[:],
                                        wslice(wsb[st * 2 + kc], nh),
                                        ysb[(m, src, kc)][:],
                                        start=(i == 0), stop=(i == len(mms) - 1),
                                    )
                                ob = osb_p.tile([128, NB], f32, tag=f"o{m}{ri_o}{nh}")
                                nc.scalar.copy(ob[:], yp[:])
                                nc.sync.dma_start(
                                    yf_d[sidx(b, m, ri_o), nh * 128:(nh + 1) * 128,
                                         j0:j0 + NB],
                                    ob[:],
                                )

    nc.compile()
    return nc


def _run_device(xf, wmat, gmat, trace=False):
    """xf [B,NM,2,NFFT,FTOT] -> yf same shape, via 8-core SPMD bass kernel."""
    global _PROG, LAST_EXEC_NS, LAST_RESULTS
    from concourse.bass_utils import run_bass_kernel_spmd

    if _PROG is None:
        _PROG = _build_program()
    nc = _PROG

    xfs = xf.reshape(NSTREAMS, NFFT, FTOT)
    in_maps = []
    for k in range(NCORES):
        in_maps.append({
            "xf": np.ascontiguousarray(xfs[:, :, k * NH:(k + 1) * NH]),
            "wmat": wmat,
            "gmat": gmat,
        })
    kwargs = {}
    if trace:
        kwargs["trace"] = True
    res = run_bass_kernel_spmd(nc, in_maps, list(range(NCORES)), **kwargs)
    LAST_EXEC_NS = res.exec_time_ns
    LAST_RESULTS = res
    yf = np.empty((NSTREAMS, NFFT, FTOT), np.float32)
    for k in range(NCORES):
        yf[:, :, k * NH:(k + 1) * NH] = res.results[k]["yf"]
    return yf.reshape(B, NM, 2, NFFT, FTOT)


def _emulate_device(xf, wmat, gmat):
    """Numpy mirror of the device program (same constants/layout)."""
    W = {st: np.concatenate([wmat[st * 2], wmat[st * 2 + 1]], 0) for st in range(6)}
    yf = np.empty_like(xf)
    for b in range(B):
        G = {ri: np.concatenate([gmat[b, ri * 2], gmat[b, ri * 2 + 1]], 0) for ri in range(2)}
        Xr = np.einsum('nf,mnj->mfj', W[0], xf[b, :, 0]) + np.einsum('nf,mnj->mfj', W[2], xf[b, :, 1])
        Xi = np.einsum('nf,mnj->mfj', W[0], xf[b, :, 1]) + np.einsum('nf,mnj->mfj', W[1], xf[b, :, 0])
        I = (Xr * Xr + Xi * Xi).sum(axis=0)
        phr = G[0].T @ I
        phi = G[1].T @ I
        A, Bm = 1.0 - phi, phr
        Yr, Yi = Xr * A - Xi * Bm, Xi * A + Xr * Bm
        yf[b, :, 0] = np.einsum('fn,mfj->mnj', W[3], Yr) + np.einsum('fn,mfj->mnj', W[5], Yi)
        yf[b, :, 1] = np.einsum('fn,mfj->mnj', W[3], Yi) + np.einsum('fn,mfj->mnj', W[4], Yr)
    return yf


def kernel(x_real, x_imag, task_info, h_real, h_imag, _emulate=False, _trace=False):
    x_real = np.asarray(x_real, np.float32)
    x_imag = np.asarray(x_imag, np.float32)
    wmat, gmat = _build_const_matrices(h_real, h_imag, task_info)
    xf = _frame_inputs(x_real, x_imag)
    if _emulate:
        yf = _emulate_device(xf, wmat, gmat)
    else:
        yf = _run_device(xf, wmat, gmat, trace=_trace)
    y = _overlap_add(yf)                      # [B, NM, 2, L]
    y = y[:, :, :, PAD:L - PAD]               # trim overlaps//2 each side
    return np.ascontiguousarray(y.transpose(0, 3, 1, 2))  # [B, LOUT, NM, 2]


# revision 6
# speedup vs baseline: 1.5757x; 1.5757x over previous
"""Trainium2 Bass kernel for nn_EqStftSnsePBC (STFT -> per-tap nonlinear PBC -> ISTFT).

Strategy (8 NeuronCores, pure data parallelism over STFT frames):
  host:   frame the signal (hop 216, n_fft 256) into [stream, freq_in_frame, frame]
          fp32 arrays, shard 4632 (padded) frames as 579 per core; build DFT/IDFT
          matrices and the 256x256 wrap-around Toeplitz correlation matrix G
          (launch power P folded in).
  device: X = DFT(frames)        (fp32r matmuls, K=256 contracted in 2 chunks)
          I = sum_modes |X|^2    (DVE)
          phi' = G @ I           (fp32r matmuls; phi' = P * phi)
          Y = X * ((1 - phi'_i) + j phi'_r)   (GPSIMD elementwise)
          yf = IDFT(Y)           (fp32r matmuls, 1/256 folded into the matrix)
  host:   overlap-add yf frames, divide by coverage count, trim, restack.
"""

import os
import sys

for _p in ("/opt/trn_rl_repo",):
    if os.path.isdir(_p) and _p not in sys.path:
        sys.path.append(_p)

import numpy as np

# ---- problem geometry (hardcoded) ----
MTAPS = 41
PAD = MTAPS // 2  # 20
NFFT = 256
HOP = 216
B = 2
NM = 2
L = 999688
STEPS = 4628            # (L - NFFT) // HOP + 1
NCORES = 8
NH = 579                # frames per core
FTOT = NCORES * NH      # 4632 >= STEPS (4 trailing fake frames, ignored on host)
LOUT = L - 2 * PAD * 2  # L - overlaps = 999648
NBLOCKS = ((0, 290), (289, 290))   # (col offset, width); even widths >= 256 (fp32r ISA); 1-col overlap is benign
NSTREAMS = B * NM * 2   # (b, mode, re/im) -> 8

_PROG = None            # (nc, input_names) cache; compile once per process
LAST_EXEC_NS = None
LAST_RESULTS = None


def _build_const_matrices(h_real, h_imag, task_info):
    """DFT/IDFT lhsT matrices and per-batch P-scaled correlation matrices."""
    n = np.arange(NFFT)
    ang = 2.0 * np.pi * np.outer(n, n) / NFFT
    c, s = np.cos(ang), np.sin(ang)
    # forward lhsT[n, f] so that lhsT.T @ frames = W @ frames, W = exp(-2i pi f n / N)
    # stages: 0 fwd_r, 1 fwd_i, 2 fwd_minus_i, 3 inv_r, 4 inv_i, 5 inv_minus_i
    wmat = np.empty((12, 128, NFFT), np.float32)
    stages = [c, -s, s, c / NFFT, s / NFFT, -s / NFFT]
    for st, mat in enumerate(stages):
        wmat[st * 2 + 0] = mat[0:128, :].astype(np.float32)
        wmat[st * 2 + 1] = mat[128:256, :].astype(np.float32)

    # G[p', m] = sum of h[p-m] over p in [0,296) with (p-20) mod 256 == p'
    def toep(h):
        G = np.zeros((NFFT, NFFT), np.float64)
        for p in range(NFFT + 2 * PAD):
            pp = (p - PAD) % NFFT
            lo, hi = max(0, p - (MTAPS - 1)), min(NFFT - 1, p)
            if lo <= hi:
                ms = np.arange(lo, hi + 1)
                G[pp, ms] += h[p - ms]
        return G

    Gr, Gi = toep(np.asarray(h_real, np.float64)), toep(np.asarray(h_imag, np.float64))
    P = (10.0 ** (np.asarray(task_info, np.float64)[:, 0] / 10.0) / NM)
    # negated so the device computes na = -P*phi_r, nb = -P*phi_i directly
    gmat = np.empty((B, 4, 128, NFFT), np.float32)
    for b in range(B):
        for kc in range(2):
            gmat[b, 0 * 2 + kc] = (-P[b] * Gr[kc * 128:(kc + 1) * 128, :]).astype(np.float32)
            gmat[b, 1 * 2 + kc] = (-P[b] * Gi[kc * 128:(kc + 1) * 128, :]).astype(np.float32)
    return wmat, gmat


def _frame_inputs(x_real, x_imag):
    """-> xf [B, NM, 2, NFFT, FTOT] fp32: xf[b,m,ri,n,j] = x[b, HOP*j + n, m]."""
    need = HOP * (FTOT - 1) + NFFT
    xf = np.empty((B, NM, 2, NFFT, FTOT), np.float32)
    for ri, x in enumerate((x_real, x_imag)):
        xt = np.ascontiguousarray(np.asarray(x).transpose(0, 2, 1))  # [B, NM, L]
        xp = np.zeros((B, NM, need), np.float32)
        xp[:, :, :L] = xt
        sw = np.lib.stride_tricks.as_strided(
            xp,
            shape=(B, NM, FTOT, NFFT),
            strides=(xp.strides[0], xp.strides[1], HOP * 4, 4),
        )
        xf[:, :, ri] = sw.transpose(0, 1, 3, 2)
    return xf


def _overlap_add(yf):
    """yf [B, NM, 2, NFFT, FTOT] -> y [B, NM, 2, L] (OLA / coverage)."""
    y = np.zeros((B, NM, 2, STEPS, HOP), np.float32)
    body = yf[:, :, :, :HOP, :STEPS].transpose(0, 1, 2, 4, 3)  # [.., j, 216]
    y[:] = body
    tail = yf[:, :, :, HOP:, :STEPS - 1].transpose(0, 1, 2, 4, 3)  # [.., j, 40]
    y[:, :, :, 1:, :NFFT - HOP] += tail
    y = y.reshape(B, NM, 2, STEPS * HOP)
    yfull = np.empty((B, NM, 2, L), np.float32)
    yfull[:, :, :, :STEPS * HOP] = y
    yfull[:, :, :, STEPS * HOP:] = yf[:, :, :, HOP:, STEPS - 1]  # final tail, coverage 1
    # coverage: 2 on [216(j+1), 216(j+1)+40) for j=0..steps-2, else 1
    t = np.arange(L)
    wsum = np.ones(L, np.float32)
    wsum[(t >= HOP) & (t < STEPS * HOP) & (t % HOP < NFFT - HOP)] = 2.0
    yfull /= wsum
    return yfull


def _build_program():
    import concourse.bass as bass
    import concourse.tile as tile
    from concourse import bacc, mybir
    from contextlib import ExitStack

    f32 = mybir.dt.float32
    f32r = mybir.dt.float32r
    bf16 = mybir.dt.bfloat16
    MULT = mybir.AluOpType.mult
    ADD = mybir.AluOpType.add
    SUB = mybir.AluOpType.subtract

    nc = bacc.Bacc(None, target_bir_lowering=False, debug=False)
    xf_d = nc.dram_tensor("xf", [NSTREAMS, NFFT, NH], f32, kind="ExternalInput").ap()
    wm_d = nc.dram_tensor("wmat", [12, 128, NFFT], f32, kind="ExternalInput").ap()
    gm_d = nc.dram_tensor("gmat", [B, 4, 128, NFFT], f32, kind="ExternalInput").ap()
    vf_d = nc.dram_tensor("vf", [NSTREAMS, NFFT, NH], bf16, kind="ExternalOutput").ap()

    FWD_R, FWD_I, FWD_MI, INV_R, INV_I, INV_MI = range(6)
    FFT_TERMS = {0: ((FWD_R, 0), (FWD_MI, 1)),   # Xr = Wr xr - Wi xi
                 1: ((FWD_R, 1), (FWD_I, 0))}    # Xi = Wr xi + Wi xr
    IFFT_TERMS = {0: ((INV_R, 0), (INV_MI, 1)),  # Vr = iWr Ur - iWi Ui
                  1: ((INV_R, 1), (INV_I, 0))}   # Vi = iWr Ui + iWi Ur

    with tile.TileContext(nc) as tc:
        with ExitStack() as ctx:
            consts = ctx.enter_context(tc.tile_pool(name="consts", bufs=1))
            xin = ctx.enter_context(tc.tile_pool(name="xin", bufs=2))
            xcp = ctx.enter_context(tc.tile_pool(name="xcp", bufs=2))
            work = ctx.enter_context(tc.tile_pool(name="work", bufs=2))
            usb_p = ctx.enter_context(tc.tile_pool(name="usb", bufs=2))
            osb_p = ctx.enter_context(tc.tile_pool(name="osb", bufs=2))
            ps_x = ctx.enter_context(tc.tile_pool(name="psx", bufs=4, space="PSUM"))
            ps_phi = ctx.enter_context(tc.tile_pool(name="psphi", bufs=2, space="PSUM"))
            ps_v = ctx.enter_context(tc.tile_pool(name="psv", bufs=2, space="PSUM"))

            # constants: forward DFT as f32r, inverse DFT + G as bf16
            wsb = {}
            for st in (FWD_R, FWD_I, FWD_MI):
                for kc in range(2):
                    for mh in range(2):
                        w = consts.tile([128, 128], f32r, tag=f"w{st}_{kc}_{mh}")
                        nc.sync.dma_start(
                            w[:], wm_d[st * 2 + kc, :, mh * 128:(mh + 1) * 128].bitcast(f32r))
                        wsb[(st, kc, mh)] = w
            for st in (INV_R, INV_I, INV_MI):
                for kc in range(2):
                    for mh in range(2):
                        w = consts.tile([128, 128], bf16, tag=f"w{st}_{kc}_{mh}")
                        nc.gpsimd.dma_start(
                            w[:], wm_d[st * 2 + kc, :, mh * 128:(mh + 1) * 128])
                        wsb[(st, kc, mh)] = w
            gsb = {}
            for b in range(B):
                for t in range(4):
                    for mh in range(2):
                        g = consts.tile([128, 128], bf16, tag=f"g{b}_{t}_{mh}")
                        nc.gpsimd.dma_start(
                            g[:], gm_d[b, t, :, mh * 128:(mh + 1) * 128])
                        gsb[(b, t, mh)] = g

            def sidx(b, m, ri):
                return b * 4 + m * 2 + ri

            for b in range(B):
                for (j0, NB) in NBLOCKS:
                    # ---- load frames (f32r, straight from HBM) ----
                    xsb = {}
                    for m in range(NM):
                        for ri in range(2):
                            for kc in range(2):
                                tl = xin.tile([128, NB], f32r, tag=f"x{m}{ri}{kc}")
                                nc.sync.dma_start(
                                    tl[:],
                                    xf_d[sidx(b, m, ri), kc * 128:(kc + 1) * 128,
                                         j0:j0 + NB].bitcast(f32r),
                                )
                                xsb[(m, ri, kc)] = tl

                    # ---- FFT (f32r), m0/m1 paired to share weights; X -> bf16 SBUF ----
                    Xsb = {}
                    for ri_o in range(2):
                        for mh in range(2):
                            xps = [ps_x.tile([128, NB], f32, tag="xps", name=f"xps{b}{j0}{ri_o}{mh}{_m}") for _m in range(NM)]
                            seq = [(st, src, kc)
                                   for (st, src) in FFT_TERMS[ri_o] for kc in range(2)]
                            for i, (st, src, kc) in enumerate(seq):
                                for m in range(NM):
                                    nc.tensor.matmul(
                                        xps[m][:], wsb[(st, kc, mh)][:],
                                        xsb[(m, src, kc)][:],
                                        start=(i == 0), stop=(i == len(seq) - 1),
                                    )
                            for m in range(NM):
                                xs = xcp.tile([128, NB], bf16, tag=f"X{m}{ri_o}{mh}")
                                nc.vector.tensor_copy(xs[:], xps[m][:])
                                Xsb[(m, ri_o, mh)] = xs

                    # ---- intensity (bf16) ----
                    isb = {}
                    for mh in range(2):
                        it = work.tile([128, NB], bf16, tag=f"i{mh}")
                        t0 = work.tile([128, NB], bf16, tag=f"sqa{mh}")
                        t1 = work.tile([128, NB], bf16, tag=f"sqb{mh}")
                        t2 = work.tile([128, NB], bf16, tag=f"sqc{mh}")
                        nc.vector.tensor_tensor(t0[:], Xsb[(0, 0, mh)][:], Xsb[(0, 0, mh)][:], MULT)
                        nc.vector.tensor_tensor(t1[:], Xsb[(0, 1, mh)][:], Xsb[(0, 1, mh)][:], MULT)
                        nc.vector.tensor_tensor(t0[:], t0[:], t1[:], ADD)
                        nc.vector.tensor_tensor(t2[:], Xsb[(1, 0, mh)][:], Xsb[(1, 0, mh)][:], MULT)
                        nc.vector.tensor_tensor(t1[:], Xsb[(1, 1, mh)][:], Xsb[(1, 1, mh)][:], MULT)
                        nc.vector.tensor_tensor(t2[:], t2[:], t1[:], ADD)
                        nc.vector.tensor_tensor(it[:], t0[:], t2[:], ADD)
                        isb[mh] = it

                    # ---- na = -P phi_r, nb = -P phi_i (bf16 matmul, copy to bf16 SBUF) ----
                    nab = {}
                    for mh in range(2):
                        for ri in range(2):  # 0 -> na (from Gr), 1 -> nb (from Gi)
                            pp = ps_phi.tile([128, NB], f32, tag="phps")
                            for kc in range(2):
                                nc.tensor.matmul(
                                    pp[:], gsb[(b, ri * 2 + kc, mh)][:], isb[kc][:],
                                    start=(kc == 0), stop=(kc == 1),
                                )
                            ab = work.tile([128, NB], bf16, tag=f"ab{ri}{mh}")
                            nc.scalar.copy(ab[:], pp[:])
                            nab[(ri, mh)] = ab

                    # ---- U = j P phi * X  (bf16):
                    #      Ur = nb*Xr + na*Xi ; Ui = nb*Xi - na*Xr ----
                    usb = {}
                    for m in range(NM):
                        eng = nc.gpsimd if m == 0 else nc.vector
                        for mh in range(2):
                            na, nb_ = nab[(0, mh)], nab[(1, mh)]
                            Xr, Xi = Xsb[(m, 0, mh)], Xsb[(m, 1, mh)]
                            t0 = work.tile([128, NB], bf16, tag=f"ut0{m}{mh}")
                            t1 = work.tile([128, NB], bf16, tag=f"ut1{m}{mh}")
                            ur = usb_p.tile([128, NB], bf16, tag=f"ur{m}{mh}")
                            ui = usb_p.tile([128, NB], bf16, tag=f"ui{m}{mh}")
                            eng.tensor_tensor(t0[:], nb_[:], Xr[:], MULT)
                            eng.tensor_tensor(t1[:], na[:], Xi[:], MULT)
                            eng.tensor_tensor(ur[:], t0[:], t1[:], ADD)
                            eng.tensor_tensor(t0[:], nb_[:], Xi[:], MULT)
                            eng.tensor_tensor(t1[:], na[:], Xr[:], MULT)
                            eng.tensor_tensor(ui[:], t0[:], t1[:], SUB)
                            usb[(m, 0, mh)] = ur
                            usb[(m, 1, mh)] = ui

                    # ---- V = IFFT(U) (bf16), m0/m1 paired; out bf16 ----
                    for ri_o in range(2):
                        for nh in range(2):
                            vps = [ps_v.tile([128, NB], f32, tag="vps", name=f"vps{b}{j0}{ri_o}{nh}{_m}") for _m in range(NM)]
                            seq = [(st, src, kc)
                                   for (st, src) in IFFT_TERMS[ri_o] for kc in range(2)]
                            for i, (st, src, kc) in enumerate(seq):
                                for m in range(NM):
                                    nc.tensor.matmul(
                                        vps[m][:], wsb[(st, kc, nh)][:],
                                        usb[(m, src, kc)][:],
                                        start=(i == 0), stop=(i == len(seq) - 1),
                                    )
                            for m in range(NM):
                                ob = osb_p.tile([128, NB], bf16, tag=f"o{m}{ri_o}{nh}")
                                nc.scalar.copy(ob[:], vps[m][:])
                                nc.scalar.dma_start(
                                    vf_d[sidx(b, m, ri_o), nh * 128:(nh + 1) * 128,
                                         j0:j0 + NB],
                                    ob[:],
                                )

    nc.compile()
    return nc


def _run_device(xf, wmat, gmat, trace=False):
    """xf [B,NM,2,NFFT,FTOT] -> yf same shape, via 8-core SPMD bass kernel."""
    global _PROG, LAST_EXEC_NS, LAST_RESULTS
    from concourse.bass_utils import run_bass_kernel_spmd

    if _PROG is None:
        _PROG = _build_program()
    nc = _PROG

    xfs = xf.reshape(NSTREAMS, NFFT, FTOT)
    in_maps = []
    for k in range(NCORES):
        in_maps.append({
            "xf": np.ascontiguousarray(xfs[:, :, k * NH:(k + 1) * NH]),
            "wmat": wmat,
            "gmat": gmat,
        })
    kwargs = {}
    if trace:
        kwargs["trace"] = True
    res = run_bass_kernel_spmd(nc, in_maps, list(range(NCORES)), **kwargs)
    LAST_EXEC_NS = res.exec_time_ns
    LAST_RESULTS = res
    vf = np.empty((NSTREAMS, NFFT, FTOT), np.float32)
    for k in range(NCORES):
        vf[:, :, k * NH:(k + 1) * NH] = res.results[k]["vf"].astype(np.float32)
    return vf.reshape(B, NM, 2, NFFT, FTOT)


def _emulate_device(xf, wmat, gmat):
    """Numpy mirror of the device program: returns V = IFFT(j P phi * X)."""
    W = {st: np.concatenate([wmat[st * 2], wmat[st * 2 + 1]], 0) for st in range(6)}
    vf = np.empty_like(xf)
    for b in range(B):
        G = {ri: np.concatenate([gmat[b, ri * 2], gmat[b, ri * 2 + 1]], 0) for ri in range(2)}
        Xr = np.einsum('nf,mnj->mfj', W[0], xf[b, :, 0]) + np.einsum('nf,mnj->mfj', W[2], xf[b, :, 1])
        Xi = np.einsum('nf,mnj->mfj', W[0], xf[b, :, 1]) + np.einsum('nf,mnj->mfj', W[1], xf[b, :, 0])
        I = (Xr * Xr + Xi * Xi).sum(axis=0)
        na = G[0].T @ I    # = -P*phi_r
        nb = G[1].T @ I    # = -P*phi_i
        Ur, Ui = nb * Xr + na * Xi, nb * Xi - na * Xr
        vf[b, :, 0] = np.einsum('fn,mfj->mnj', W[3], Ur) + np.einsum('fn,mfj->mnj', W[5], Ui)
        vf[b, :, 1] = np.einsum('fn,mfj->mnj', W[3], Ui) + np.einsum('fn,mfj->mnj', W[4], Ur)
    return vf


def kernel(x_real, x_imag, task_info, h_real, h_imag, _emulate=False, _trace=False):
    x_real = np.asarray(x_real, np.float32)
    x_imag = np.asarray(x_imag, np.float32)
    wmat, gmat = _build_const_matrices(h_real, h_imag, task_info)
    xf = _frame_inputs(x_real, x_imag)
    if _emulate:
        vf = _emulate_device(xf, wmat, gmat)
    else:
        vf = _run_device(xf, wmat, gmat, trace=_trace)
    yf = xf + vf                              # exact passthrough + device correction
    y = _overlap_add(yf)                      # [B, NM, 2, L]
    y = y[:, :, :, PAD:L - PAD]               # trim overlaps//2 each side
    return np.ascontiguousarray(y.transpose(0, 3, 1, 2))  # [B, LOUT, NM, 2]
